# revision 25
# baseline (speedup 1.0000x reference)
# MoE top-2 routing kernel for Trainium2, 8 NeuronCores, data-parallel over batch.
#
# Problem (hardcoded): x[8,2048,512] f32, router Wg[512,8]+bg, 8 experts
#   W1[8,512,768], b1[8,768], W2[8,768,512], b2[8,512];
#   out = sum_{k in top2} gate_k * (GELU(x@W1[e_k]+b1[e_k])@W2[e_k]+b2[e_k])
#
# Strategy per core (1 batch row = 2048 tokens), all feature-major ([E,T] /
# [H,T] layouts, x pre-transposed on host so no on-device data transposes):
#   1. fp32 router: scores on PE, top-2 via DVE max8 (tie semantics match
#      jax.lax.top_k), gates via sigmoid closed form
#   2. combine weights c[t,e] = gate if e in top2 else 0; transposed to
#      expert-major rows [8,T] via PE, staged in DRAM, and broadcast to all
#      128 partitions by stride-0 DMA reads
#   3. dense expert sweep in bf16: for each 512-token chunk, all 8 experts:
#      h = W1e^T @ xT -> GELU(+b1) -> scale by c row -> accumulate
#      W2e^T @ g over all experts directly in PSUM; b2 enters as a tiny
#      K=8 matmul (b2^T @ c_rows) into the same PSUM accumulator
#   4. output written feature-major [E,T]; host transposes back to [T,E]
import numpy as np
import ml_dtypes

B, N, E, H, X = 8, 2048, 512, 768, 8
T = N
P = 128
NT = T // P            # 16 token tiles
KE = E // P            # 4
KH = H // P            # 6
TC = 512               # token chunk for expert sweep
NC_ = T // TC          # 4 chunks

bf16 = ml_dtypes.bfloat16

_PROGRAM_CACHE = {}
SWEEP_REPEAT = 1   # >1 only for timing experiments (repeats the expert sweep)


def build_program():
    import concourse.bass as bass
    import concourse.mybir as mybir
    import concourse.tile as tile
    from concourse import bacc
    from concourse.tile import add_dep_helper

    f32 = mybir.dt.float32
    bf = mybir.dt.bfloat16
    Alu = mybir.AluOpType
    Act = mybir.ActivationFunctionType

    nc = bacc.Bacc()

    xT = nc.dram_tensor("xT", [E, T], f32, kind="ExternalInput")
    wg = nc.dram_tensor("wg", [P, KE, X], f32, kind="ExternalInput")
    w1 = nc.dram_tensor("w1", [P, X, KE, H], bf, kind="ExternalInput")
    w2 = nc.dram_tensor("w2", [P, X, KH, E], bf, kind="ExternalInput")
    bgb = nc.dram_tensor("bgb", [P, X], f32, kind="ExternalInput")
    b1p = nc.dram_tensor("b1p", [P, X, KH], f32, kind="ExternalInput")
    b2r = nc.dram_tensor("b2r", [X, E], bf, kind="ExternalInput")
    out = nc.dram_tensor("out", [E, T], f32, kind="ExternalOutput")

    identb = nc.inline_tensor(np.eye(P).astype(bf16), "identb")
    sel_np = np.zeros((X, X * P), dtype=np.float32)
    for _e in range(X):
        sel_np[_e, _e * P : (_e + 1) * P] = 1.0
    sel8 = nc.inline_tensor(sel_np.astype(bf16), "sel8")
    iota8 = nc.inline_tensor(
        np.tile(np.arange(X, dtype=np.float32), (P, 1)), "iota8"
    )


    with tile.TileContext(nc) as tc, tc.tile_pool(name="persist", bufs=1) as persist:
        # ---- persistent tiles; router-critical loads issued first so the
        # PE's first score matmul is not stuck behind 13MB of weights ----
        wgp_sb = persist.tile([P, KE, X], f32)
        nc.gpsimd.dma_start(out=wgp_sb[:], in_=wg[:])
        io_sb = persist.tile([P, X], f32)
        nc.gpsimd.dma_start(out=io_sb[:], in_=iota8[:])
        bgp_sb = persist.tile([P, X], f32)
        nc.gpsimd.dma_start(out=bgp_sb[:], in_=bgb[:])
        idb_sb = persist.tile([P, P], bf)
        nc.gpsimd.dma_start(out=idb_sb[:], in_=identb[:])
        b1_sb = persist.tile([P, X, KH], f32)
        nc.gpsimd.dma_start(out=b1_sb[:], in_=b1p[:])
        b2_sb = persist.tile([X, E], bf)
        nc.gpsimd.dma_start(out=b2_sb[:], in_=b2r[:])
        sel_sb = persist.tile([X, X * P], bf)
        nc.gpsimd.dma_start(out=sel_sb[:], in_=sel8[:])
        cem = persist.tile([8, T], bf)

        cw_insts = []
        exp_ctx = (
            tc.tile_pool(name="exp", bufs=2),
            tc.tile_pool(name="outp", bufs=2),
            tc.tile_pool(name="psh", bufs=2, space="PSUM"),
            tc.tile_pool(name="pso", bufs=1, space="PSUM"),
            tc.tile_pool(name="pbc", bufs=2, space="PSUM"),
        )
        expp = exp_ctx[0].__enter__()
        outp = exp_ctx[1].__enter__()
        psh = exp_ctx[2].__enter__()
        pso = exp_ctx[3].__enter__()
        pbc = exp_ctx[4].__enter__()
        with (
            tc.tile_pool(name="router", bufs=1) as router,
            tc.tile_pool(name="rsmall", bufs=2) as rsmall,
        ):
            # ---- router (fp32) ----
            xT_v = xT.rearrange("(k p) t -> p k t", p=P)
            xT_sb = router.tile([P, KE, T], f32)
            for q in range(8):
                qs = q * (T // 8)
                nc.sync.dma_start(
                    out=xT_sb[:, :, qs : qs + T // 8],
                    in_=xT_v[:, :, qs : qs + T // 8],
                )
            wg_sb = wgp_sb
            bg_sb = bgp_sb

            s_all = router.tile([P, NT, X], f32)
            mx_all = router.tile([P, NT, 8], f32)
            xTb_c = []
            for q in range(NC_):
                xc = persist.tile([P, KE, TC], bf, name=f"xTb_c{q}", tag=f"xTb_c{q}")
                xTb_c.append(xc)
                nc.vector.tensor_copy(
                    out=xc[:], in_=xT_sb[:, :, q * TC : (q + 1) * TC]
                )
            w1_e, w2_e = [], []
            for e in range(X):
                w1_e.append(persist.tile([P, KE, H], bf, name=f"w1e{e}", tag=f"w1e{e}"))
                w2_e.append(persist.tile([P, KH, E], bf, name=f"w2e{e}", tag=f"w2e{e}"))
            nc.sync.dma_start(out=w1_e[0][:], in_=w1[:, 0, :, :])
            nc.sync.dma_start(out=w2_e[0][:], in_=w2[:, 0, :, :])
            for e in range(1, X):
                nc.sync.dma_start(out=w1_e[e][:], in_=w1[:, e, :, :])
                nc.sync.dma_start(out=w2_e[e][:], in_=w2[:, e, :, :])
            for tt in range(NT):
                ps = pbc.tile([P, X], f32, tag="bc", name=f"ps{tt}")
                for k in range(KE):
                    nc.tensor.matmul(
                        ps[:],
                        lhsT=xT_sb[:, k, tt * P : (tt + 1) * P],
                        rhs=wg_sb[:, k, :],
                        start=(k == 0),
                        stop=(k == KE - 1),
                    )
                nc.vector.tensor_tensor(
                    out=s_all[:, tt, :], in0=ps[:], in1=bg_sb[:], op=Alu.add
                )
                nc.vector.max(out=mx_all[:, tt, :], in_=s_all[:, tt, :])

            iob = io_sb[:, None, :].to_broadcast([P, NT, X])
            m1b = mx_all[:, :, 0:1].to_broadcast([P, NT, X])
            m2b = mx_all[:, :, 1:2].to_broadcast([P, NT, X])

            # top-1 one-hot (min index among score==max, matching top_k ties)
            mask0 = router.tile([P, NT, X], f32)
            nc.vector.tensor_tensor(out=mask0[:], in0=s_all[:], in1=m1b, op=Alu.is_ge)
            tsel = router.tile([P, NT, X], f32)
            nc.vector.scalar_tensor_tensor(
                out=tsel[:], in0=mask0[:], scalar=float(X), in1=iob,
                op0=Alu.mult, op1=Alu.subtract,
            )
            e0n = router.tile([P, NT, 1], f32)
            nc.vector.tensor_reduce(
                out=e0n[:], in_=tsel[:], op=Alu.max, axis=mybir.AxisListType.X
            )
            e0 = router.tile([P, NT, 1], f32)
            nc.vector.tensor_scalar(
                out=e0[:], in0=e0n[:], scalar1=-1.0, scalar2=float(X),
                op0=Alu.mult, op1=Alu.add,
            )
            oh0 = router.tile([P, NT, X], f32)
            nc.vector.tensor_tensor(
                out=oh0[:], in0=iob, in1=e0[:, :, 0:1].to_broadcast([P, NT, X]),
                op=Alu.is_equal,
            )
            # top-2 one-hot: min index among (s >= second max) excluding e0
            mask2 = router.tile([P, NT, X], f32)
            nc.vector.tensor_tensor(out=mask2[:], in0=s_all[:], in1=m2b, op=Alu.is_ge)
            nc.vector.tensor_tensor(out=mask2[:], in0=mask2[:], in1=oh0[:], op=Alu.subtract)
            nc.vector.scalar_tensor_tensor(
                out=tsel[:], in0=mask2[:], scalar=float(X), in1=iob,
                op0=Alu.mult, op1=Alu.subtract,
            )
            e1n = router.tile([P, NT, 1], f32)
            nc.vector.tensor_reduce(
                out=e1n[:], in_=tsel[:], op=Alu.max, axis=mybir.AxisListType.X
            )
            e1 = router.tile([P, NT, 1], f32)
            nc.vector.tensor_scalar(
                out=e1[:], in0=e1n[:], scalar1=-1.0, scalar2=float(X),
                op0=Alu.mult, op1=Alu.add,
            )
            oh1 = router.tile([P, NT, X], f32)
            nc.vector.tensor_tensor(
                out=oh1[:], in0=iob, in1=e1[:, :, 0:1].to_broadcast([P, NT, X]),
                op=Alu.is_equal,
            )

            # gates: softmax of the two selected logits
            c0_all = router.tile([P, NT, 1], f32)
            d01 = rsmall.tile([P, NT, 1], f32)
            nc.vector.tensor_tensor(
                out=d01[:], in0=mx_all[:, :, 0:1], in1=mx_all[:, :, 1:2], op=Alu.subtract
            )
            nc.scalar.activation(out=c0_all[:], in_=d01[:], func=Act.Sigmoid)
            c1_all = router.tile([P, NT, 1], f32)
            nc.vector.tensor_scalar(
                out=c1_all[:], in0=c0_all[:], scalar1=-1.0, scalar2=1.0,
                op0=Alu.mult, op1=Alu.add,
            )

            # combine weights c[t,e] = c0*oh0 + c1*oh1, in bf16
            ctok = router.tile([P, NT, X], f32)
            nc.vector.tensor_tensor(
                out=ctok[:], in0=oh0[:],
                in1=c0_all[:, :, 0:1].to_broadcast([P, NT, X]), op=Alu.mult
            )
            ctmp = router.tile([P, NT, X], f32)
            nc.vector.tensor_tensor(
                out=ctmp[:], in0=oh1[:],
                in1=c1_all[:, :, 0:1].to_broadcast([P, NT, X]), op=Alu.mult
            )
            nc.vector.tensor_tensor(out=ctok[:], in0=ctok[:], in1=ctmp[:], op=Alu.add)
            ctok_b = router.tile([P, NT, X], bf)
            nc.vector.tensor_copy(out=ctok_b[:], in_=ctok[:])

            # transpose to expert-major rows [8, T]
            for tt in range(NT):
                pt = pbc.tile([X, P], bf, tag="bc", name=f"pt{tt}")
                nc.tensor.transpose(
                    out=pt[:], in_=ctok_b[:, tt, :], identity=idb_sb[:]
                )
                nc.scalar.copy(out=cem[:, tt * P : (tt + 1) * P], in_=pt[:])

        # ---- dense expert sweep, bf16, PSUM-accumulated over experts ----
        if True:
            for ch in range(NC_ * SWEEP_REPEAT):
                rep_last = ch >= NC_ * (SWEEP_REPEAT - 1)
                ch = ch % NC_
                t0 = ch * TC
                po = [pso.tile([P, TC], f32, tag=f"po{es}", name=f"po{es}") for es in range(KE)]
                for e in range(X):
                    # broadcast this expert's gate row across 128 partitions:
                    # K=8 one-hot matmul from the expert-major cem rows
                    pcb = pbc.tile([P, TC], f32, tag="bc", name=f"pcb{ch}_{e}")
                    nc.tensor.matmul(
                        pcb[:],
                        lhsT=sel_sb[:, e * P : (e + 1) * P],
                        rhs=cem[:, t0 : t0 + TC],
                        start=True,
                        stop=True,
                    )
                    cb = expp.tile([P, TC], bf, tag="cb")
                    nc.scalar.copy(out=cb[:], in_=pcb[:])
                    g = expp.tile([P, KH, TC], bf, tag="g")
                    for hs in range(KH):
                        ph = psh.tile([P, TC], f32, tag="ph")
                        for k in range(KE):
                            nc.tensor.matmul(
                                ph[:],
                                lhsT=w1_e[e][:, k, hs * P : (hs + 1) * P],
                                rhs=xTb_c[ch][:, k, :],
                                start=(k == 0),
                                stop=(k == KE - 1),
                            )
                        nc.scalar.activation(
                            out=g[:, hs, :], in_=ph[:], func=Act.Gelu,
                            bias=b1_sb[:, e, hs : hs + 1],
                        )
                        nc.vector.tensor_tensor(
                            out=g[:, hs, :], in0=g[:, hs, :], in1=cb[:], op=Alu.mult
                        )
                    for es in range(KE):
                        for hs in range(KH):
                            nc.tensor.matmul(
                                po[es][:],
                                lhsT=w2_e[e][:, hs, es * P : (es + 1) * P],
                                rhs=g[:, hs, :],
                                start=(e == 0 and hs == 0),
                                stop=False,
                            )
                # b2 contribution: b2^T @ c_rows, K=8 matmul into the same PSUM
                for es in range(KE):
                    nc.tensor.matmul(
                        po[es][:],
                        lhsT=b2_sb[:, es * P : (es + 1) * P],
                        rhs=cem[:, t0 : t0 + TC],
                        start=False,
                        stop=True,
                    )
                    if rep_last:
                        ot = outp.tile([P, TC], f32, tag="ot")
                        nc.scalar.copy(out=ot[:], in_=po[es][:])
                        nc.sync.dma_start(
                            out=out.rearrange("(es p) t -> p es t", p=P)[
                                :, es, t0 : t0 + TC
                            ],
                            in_=ot[:],
                        )

        for cm in reversed(exp_ctx):
            cm.__exit__(None, None, None)

    nc.compile()
    return nc


def _prep_inputs(x, Wg, bg, W1, b1, W2, b2):
    """Host-side shard + relayout. Returns per-core input maps."""
    x = np.asarray(x, dtype=np.float32)
    Wg = np.asarray(Wg, dtype=np.float32)
    bg = np.asarray(bg, dtype=np.float32)
    W1 = np.asarray(W1, dtype=np.float32)
    b1 = np.asarray(b1, dtype=np.float32)
    W2 = np.asarray(W2, dtype=np.float32)
    b2 = np.asarray(b2, dtype=np.float32)

    wg_p = np.ascontiguousarray(Wg.reshape(KE, P, X).transpose(1, 0, 2))
    w1_p = np.ascontiguousarray(
        W1.reshape(X, KE, P, H).transpose(2, 0, 1, 3)
    ).astype(bf16)
    w2_p = np.ascontiguousarray(
        W2.reshape(X, KH, P, E).transpose(2, 0, 1, 3)
    ).astype(bf16)
    bg_b = np.ascontiguousarray(np.broadcast_to(bg, (P, X)))
    b1_p = np.ascontiguousarray(b1.reshape(X, KH, P).transpose(2, 0, 1))
    b2_r = b2.astype(bf16)

    in_maps = []
    for c in range(B):
        xt = np.ascontiguousarray(x[c].T)
        in_maps.append(
            {
                "xT": xt,
                "wg": wg_p,
                "w1": w1_p,
                "w2": w2_p,
                "bgb": bg_b,
                "b1p": b1_p,
                "b2r": b2_r,
            }
        )
    return in_maps


def kernel(x, Wg, bg, W1, b1, W2, b2, _trace=False):
    from concourse.bass_utils import run_bass_kernel_spmd

    if "nc" not in _PROGRAM_CACHE:
        _PROGRAM_CACHE["nc"] = build_program()
    nc = _PROGRAM_CACHE["nc"]

    in_maps = _prep_inputs(x, Wg, bg, W1, b1, W2, b2)
    res = run_bass_kernel_spmd(nc, in_maps, list(range(B)), trace=_trace)
    _PROGRAM_CACHE["last_result"] = res
    out = np.stack(
        [np.asarray(res.results[c]["out"]).T for c in range(B)], axis=0
    )
    return np.ascontiguousarray(out, dtype=np.float32)
